# revision 44
# baseline (speedup 1.0000x reference)
"""Trainium2 Bass kernel for the branched cross-attention processor.

Problem (full shapes):
  hidden_states [4, 4096, 1280], encoder_hidden_states [4, 77, 2048],
  id_embedding [2, 32, 2048], Wq/Wout [1280,1280], Wk/Wv/Wid_k/Wid_v
  [2048,1280], bout [1280].  20 heads, dh=64.  Output [4, 4096, 1280].

Sharding: data-parallel over (batch, seq-half): core c handles batch c//2,
query rows (c%2)*2048 : (c%2+1)*2048.  K/V (109 keys) are computed
per-core for its batch.  All queries are independent (full cross
attention), so no collectives are needed.

Schedule (single continuous PE stream to keep the HAM clock gate warm):
  - DMA priority order: Wq slabs interleaved with hsT blocks first (the q
    stream), KV-projection weight half-slabs behind them, Wout last.
    Every weight group is one contiguous dma_start (the runtime shards
    each DMA across all 16 engines, so big transfers lose no parallelism
    and cut sync-engine issue time 16x).
  - Q projection groups j=0..9 with one KV chunk interleaved after each
    of groups 2..9; the last two KV chunks (v columns 768:1280) run
    inside attention chunk 0, overlapping their weight DMA tail.
  - kT transposes issued as soon as their k columns are final.
  - Attention processed chunk-major (4 chunks x 512 queries x 10 head
    pairs), software-pipelined (scores+exp of pair p overlap PV/denom/
    normalize of pair p-1), with out-projection units of chunk c-1
    interleaved between pairs so the PE never idles long enough for the
    HAM clock gate to drop to 1.2GHz.
  - Output written fp16 (host upcasts); halves output DMA traffic.

Per (head,chunk) math (identical numerics to the reference):
  scoresT = kT_h^T @ qT_h                [128 keys, 512 q]
  probsT  = exp(0.125*scoresT + gapbias) (gap rows 77:96 -> 0)
  attnT   = v_h^T @ probsT               [64, 512]
  denom   = ones^T @ probsT              (replicated over the head's rows)
  attnT  *= 1/denom
  out     = attnT^T @ Wout + bout
"""

import sys
import types

import numpy as np

# ---------------------------------------------------------------------------
# problem constants (hardcoded; kernel.py must be self-contained)
# ---------------------------------------------------------------------------
B = 4
S = 4096
H = 1280
C = 2048
TE = 77          # encoder tokens
TI = 32          # id tokens
HEADS = 20
DH = 64          # head dim
P = 128
LP = 128         # padded key count: [0:77]=ehs, [77:96]=gap, [96:128]=id
GAP0, GAP1 = TE, P - TI   # 77, 96
SC = 2048        # seq rows per core
NJ = H // P      # 10
NI = C // P      # 16
NCH = SC // 512  # 4 sq-chunks of 512
NT = SC // P     # 16 sq-tiles of 128
SCALE = 1.0 / 8.0
NCORES = 8
MCHUNKS = [(0, 512), (512, 512), (1024, 256)]

# kv chunk t=(proj, n) computes columns 512n:512n+512 of the [k|v] concat
# ([*, 2560]); proj 0 = encoder tokens (rows 0:77 + zero gap), proj 1 = id
# tokens (rows 96:128).  k = cols 0:1280, v = rest.  k chunks first so the
# kT transposes can start early; chunks 8, 9 run inside attention chunk 0.
KV_PLAN = [(0, 0), (0, 1), (1, 0), (0, 2), (1, 1), (1, 2),
           (0, 3), (1, 3), (0, 4), (1, 4)]
# kT transpose groups: after KV_PLAN index -> list of kT blocks final
KT_AT = {2: [0, 1, 2, 3], 4: [4, 5, 6, 7], 5: [8, 9]}
# kv chunks interleaved after q_group j (indices into KV_PLAN)
KV_AT_Q = {2: [0], 3: [1], 4: [2], 5: [3], 6: [4], 7: [5], 8: [6], 9: [7]}
# kv chunks interleaved inside attention chunk 0, after pair hp
KV_AT_A0 = {0: [8], 2: [9]}

_NC_CACHE = {}


def _ensure_axon_hooks():
    """The image's antenv lacks axon_hooks; synthesize it so NTFF profiling
    (trace=True) works when test.py asks for it.  Harmless if unused."""
    if "antenv.axon_hooks" in sys.modules:
        return
    try:
        import antenv
        from trn_agent_boot.trn_boot import _ntff_profile_via_ctypes

        hook = _ntff_profile_via_ctypes("/opt/axon/libaxon_pjrt.so")
        m = types.ModuleType("antenv.axon_hooks")
        m.get_axon_ntff_profile_hook = lambda: hook
        m.set_axon_ntff_profile_hook = lambda h: None
        sys.modules["antenv.axon_hooks"] = m
        antenv.axon_hooks = m
    except Exception:
        pass


def build_nc():
    """Build + compile the per-core Bass program (SPMD: same NEFF, 8 cores)."""
    if "nc" in _NC_CACHE:
        return _NC_CACHE["nc"]

    import concourse.bass as bass
    import concourse.tile as tile
    from concourse import bacc, mybir
    from concourse.bass import ts

    F32 = mybir.dt.float32
    F16 = mybir.dt.float16
    R = mybir.dt.float16      # matmul operand dtype
    EXP = mybir.ActivationFunctionType.Exp

    nc = bacc.Bacc("TRN2", target_bir_lowering=False, debug=False, num_devices=NCORES)

    hsT = nc.dram_tensor("hsT", [NJ, P, SC], R, kind="ExternalInput").ap()
    xkvTp = nc.dram_tensor("xkvTp", [P, NI * LP], R, kind="ExternalInput").ap()
    wqp = nc.dram_tensor("wqp", [NJ, P, NJ * P], R, kind="ExternalInput").ap()
    # kv weights as 20 half-slabs: [2t+h] = chunk t, contraction half h
    wkvp = nc.dram_tensor("wkvp", [20, P, 8 * 512], R, kind="ExternalInput").ap()
    woutT = nc.dram_tensor("woutT", [H, H], R, kind="ExternalInput").ap()
    boutb = nc.dram_tensor("boutb", [P, H], F32, kind="ExternalInput").ap()
    out = nc.dram_tensor("out", [SC, H], F16, kind="ExternalOutput").ap()

    with tile.TileContext(nc) as tc:
        with (
            tc.tile_pool(name="pers", bufs=1) as pers,
            tc.tile_pool(name="wkvs", bufs=5) as wkvs,
        ):
            # ---- persistent constants / arrays --------------------------------
            ones_mat = pers.tile([P, DH], R, tag="ones_mat")
            nc.vector.memset(ones_mat[:, :], 1.0)
            bias_col = pers.tile([P, 1], F32, tag="bias_col")
            # engine ops need 32-aligned start partitions: write the gap
            # as [64:96] then restore [64:77]; later writes overwrite cleanly.
            nc.vector.memset(bias_col[:, :], 0.0)
            nc.vector.memset(bias_col[64:GAP1, :], -1e30)
            nc.vector.memset(bias_col[64:GAP0, :], 0.0)
            kT_sb = [pers.tile([P, LP], R, tag=f"kT{j}", name=f"kT{j}") for j in range(NJ)]
            v_sb = pers.tile([LP, HEADS * DH], R, tag="v")
            qT_sb = [pers.tile([P, SC], R, tag=f"qT{j}", name=f"qT{j}") for j in range(NJ)]
            xkvT_sb = pers.tile([P, NI * LP], R, tag="xkvT")

            wkv_t = [None] * 20

            def fetch_kv_half(th):
                if th >= 20 or wkv_t[th] is not None:
                    return
                wkv_t[th] = wkvs.tile([P, 8 * 512], R, tag="wkv", name="wkv_t")
                nc.sync.dma_start(out=wkv_t[th][:, :], in_=wkvp[th])

            kv_ps_pool = [None]  # set per phase (pskv, then psa)
            kv_ps_tag = [None]

            def kv_chunk(t):
                proj, n = KV_PLAN[t]
                ps = kv_ps_pool[0].tile([P, 512], F32, tag=kv_ps_tag[0], name="kvps")
                for h in range(2):
                    for i in range(8):
                        nc.tensor.matmul(
                            ps[:, :], xkvT_sb[:, ts(8 * h + i, LP)],
                            wkv_t[2 * t + h][:, ts(i, 512)],
                            start=(h == 0 and i == 0), stop=(h == 1 and i == 7),
                        )
                lo, hi = (0, P) if proj == 0 else (GAP1, P)
                if n < 2:
                    nc.scalar.copy(kTMP[lo:hi, ts(n, 512)], ps[lo:hi, :])
                elif n == 2:
                    nc.scalar.copy(kTMP[lo:hi, 1024:1280], ps[lo:hi, 0:256])
                    nc.scalar.copy(v_sb[lo:hi, 0:256], ps[lo:hi, 256:512])
                else:
                    v0 = 512 * n - 1280
                    nc.scalar.copy(v_sb[lo:hi, v0:v0 + 512], ps[lo:hi, :])
                for j in KT_AT.get(t, []):
                    nc.sync.dma_start(out=kT_sb[j][:, :],
                                      in_=kTMP[:, ts(j, P)], transpose=True)

            # ---- phase Q: q projection + k/v projection, interleaved ----------
            with (
                tc.tile_pool(name="phq", bufs=1) as phq,
                tc.tile_pool(name="wqs", bufs=10) as wqs,
                tc.tile_pool(name="psq", bufs=8, space="PSUM") as psq,
            ):
                hsT_sb = [phq.tile([P, SC], R, tag=f"hsT{i}", name=f"hsT{i}")
                          for i in range(NJ)]
                kTMP = phq.tile([P, H], R, tag="kTMP")

                wq_t = [None] * NJ

                def fetch_wq(j):
                    wq_t[j] = wqs.tile([P, NJ * P], R, tag="wq", name="wq_t")
                    nc.sync.dma_start(out=wq_t[j][:, :], in_=wqp[j])

                # DMA priority order: the q stream first, kv weights behind.
                # wq0/wq1 heads and hsT block 0 are split so the fused
                # group pair's first matmuls gate on ~32-160KB transfers
                # instead of ~850KB.
                for j in (0, 1):
                    wq_t[j] = wqs.tile([P, NJ * P], R, tag="wq", name="wq_t")
                nc.sync.dma_start(out=wq_t[0][:, 0:P], in_=wqp[0][:, 0:P])
                nc.sync.dma_start(out=hsT_sb[0][:, 0:512], in_=hsT[0][:, 0:512])
                nc.sync.dma_start(out=hsT_sb[0][:, 512:SC], in_=hsT[0][:, 512:SC])
                nc.sync.dma_start(out=wq_t[1][:, 0:P], in_=wqp[1][:, 0:P])
                nc.sync.dma_start(out=wq_t[0][:, P:NJ * P], in_=wqp[0][:, P:NJ * P])
                nc.sync.dma_start(out=wq_t[1][:, P:NJ * P], in_=wqp[1][:, P:NJ * P])
                wq_sched = {2: [2], 3: [3], 5: [4], 7: [5],
                            9: [6, 7, 8, 9]}
                for i in range(1, NJ):
                    for j in wq_sched.get(i, []):
                        fetch_wq(j)
                    nc.sync.dma_start(out=hsT_sb[i][:, :], in_=hsT[i])
                nc.sync.dma_start(out=xkvT_sb[:, :], in_=xkvTp)
                fetch_kv_half(0)
                fetch_kv_half(1)
                fetch_kv_half(2)

                def q_group(j):
                    pss = [psq.tile([P, 512], F32, tag="qps", name="qps")
                           for _ in range(NCH)]
                    for i in range(NJ):
                        for c in range(NCH):
                            nc.tensor.matmul(
                                pss[c][:, :], wq_t[j][:, ts(i, P)],
                                hsT_sb[i][:, ts(c, 512)],
                                start=(i == 0), stop=(i == NJ - 1),
                            )
                    for c in range(NCH):
                        nc.scalar.copy(qT_sb[j][:, ts(c, 512)], pss[c][:, :])

                def q_group_pair(j0, j1):
                    # groups j0/j1 fused, i-major: each arriving hsT block
                    # feeds 8 matmuls instead of 4, so the PE keeps pace
                    # with the hsT DMA stream during the startup ramp.
                    pss = {j: [psq.tile([P, 512], F32, tag="qps", name="qps")
                               for _ in range(NCH)] for j in (j0, j1)}
                    for i in range(NJ):
                        for j in (j0, j1):
                            for c in range(NCH):
                                nc.tensor.matmul(
                                    pss[j][c][:, :], wq_t[j][:, ts(i, P)],
                                    hsT_sb[i][:, ts(c, 512)],
                                    start=(i == 0), stop=(i == NJ - 1),
                                )
                    for j in (j0, j1):
                        for c in range(NCH):
                            # alternate engines so the 8 psum copies drain
                            # 2x faster and free banks for group 2
                            if c % 2 == 0:
                                nc.scalar.copy(qT_sb[j][:, ts(c, 512)], pss[j][c][:, :])
                            else:
                                nc.vector.tensor_copy(qT_sb[j][:, ts(c, 512)],
                                                      pss[j][c][:, :])

                kv_ps_pool[0], kv_ps_tag[0] = psq, "qps"
                q_group_pair(0, 1)
                for j in range(2, NJ):
                    q_group(j)
                    for t in KV_AT_Q.get(j, []):
                        fetch_kv_half(2 * t + 3)
                        fetch_kv_half(2 * t + 4)
                        kv_chunk(t)

            # ---- attention + out-projection, interleaved (right-side pools) ---
            attnp_cm = tc.tile_pool(name="attnp", bufs=1, side="right")
            attnp = attnp_cm.__enter__()
            attnT_sb = [attnp.tile([P, SC], R, tag=f"attnT{d}", name=f"attnT{d}")
                        for d in range(NJ)]
            boutb_sb = attnp.tile([P, H], F32, tag="boutb")
            wout_sb = [attnp.tile([P, H], R, tag=f"wout{i}", name=f"wout{i}")
                       for i in range(NJ)]
            nc.sync.dma_start(out=boutb_sb[:, :], in_=boutb)
            for i in range(NJ):
                nc.sync.dma_start(out=wout_sb[i][:, :], in_=woutT[ts(i, P), :])

            pha_cm = tc.tile_pool(name="pha", bufs=3, side="right")
            pha = pha_cm.__enter__()
            fino_cm = tc.tile_pool(name="fino", bufs=3, side="right")
            fino = fino_cm.__enter__()
            psa_cm = tc.tile_pool(name="psa", bufs=1, space="PSUM")
            psa = psa_cm.__enter__()
            pso_cm = tc.tile_pool(name="pso", bufs=1, space="PSUM")
            pso = pso_cm.__enter__()

            astate = {}

            def attn_front(c, hp):
                pts = []
                for s in range(2):
                    rq = DH * s
                    ps_s = psa.tile([P, 512], F32, tag=f"sps{s}", name="sps")
                    nc.tensor.matmul(
                        ps_s[:, :], kT_sb[hp][rq:rq + DH, :],
                        qT_sb[hp][rq:rq + DH, ts(c, 512)],
                        start=True, stop=True,
                    )
                    pts.append(ps_s)
                probs = []
                for s in range(2):
                    probsT = pha.tile([P, 512], R, tag="probsT", name="probsT")
                    nc.scalar.activation(
                        probsT[:, :], pts[s][:, :], EXP,
                        bias=bias_col[:, :], scale=SCALE,
                    )
                    probs.append(probsT)
                astate[(c, hp)] = probs

            def attn_back(c, hp):
                probs = astate.pop((c, hp))
                ps_o = psa.tile([P, 512], F32, tag="ops", name="ops")
                ps_d = psa.tile([P, 512], F32, tag="dps", name="dps")
                for s in range(2):
                    h = 2 * hp + s
                    rq = DH * s
                    nc.tensor.matmul(
                        ps_o[rq:rq + DH, :], v_sb[:, ts(h, DH)], probs[s][:, :],
                        start=True, stop=True,
                    )
                    nc.tensor.matmul(
                        ps_d[rq:rq + DH, :], ones_mat[:, :], probs[s][:, :],
                        start=True, stop=True,
                    )
                bc_sb = pha.tile([P, 512], F32, tag="bc", name="bc_sb")
                nc.vector.reciprocal_approx_fast(bc_sb[:, :], ps_d[:, :])
                nc.vector.tensor_mul(
                    attnT_sb[hp][:, ts(c, 512)], ps_o[:, :], bc_sb[:, :]
                )

            fin_t = {}
            psf_t = {}

            def out_unit(t, i):
                # out-projection for seq tile t, contraction block i
                if i == 0:
                    fin_t[t] = fino.tile([P, H], F16, tag="fin", name="fin")
                    psf_t[t] = [
                        pso.tile([P, mw], F32, tag=f"psf{m}", name="psf")
                        for m, (m0, mw) in enumerate(MCHUNKS)
                    ]
                for m, (m0, mw) in enumerate(MCHUNKS):
                    nc.tensor.matmul(
                        psf_t[t][m][:, :], attnT_sb[i][:, ts(t, P)],
                        wout_sb[i][:, m0:m0 + mw],
                        start=(i == 0), stop=(i == NJ - 1),
                    )
                if i == NJ - 1:
                    fin = fin_t.pop(t)
                    psf = psf_t.pop(t)
                    for m, (m0, mw) in enumerate(MCHUNKS):
                        nc.vector.tensor_add(
                            fin[:, m0:m0 + mw], psf[m][:, :],
                            boutb_sb[:, m0:m0 + mw]
                        )
                    nc.sync.dma_start(out=out[ts(t, P), :], in_=fin[:, :])

            # software pipeline over pairs, with out-proj units of the
            # previous chunk (4 per pair) interleaved to keep PE dense.
            pairs = [(c, hp) for c in range(NCH) for hp in range(NJ)]
            units = []  # (t, i) out-proj work queue, filled per chunk

            def interleave_units(k):
                for _ in range(k):
                    if units:
                        out_unit(*units.pop(0))

            kv_ps_pool[0], kv_ps_tag[0] = psa, "kvo"
            for idx, (c, hp) in enumerate(pairs):
                if hp == 0 and c >= 1:
                    # queue out-proj for the 4 seq tiles of chunk c-1
                    units.extend([(t, i) for t in range(4 * (c - 1), 4 * c)
                                  for i in range(NJ)])
                attn_front(c, hp)
                if idx >= 1:
                    attn_back(*pairs[idx - 1])
                if c == 0:
                    for t in KV_AT_A0.get(hp, []):
                        fetch_kv_half(2 * t + 2)
                        fetch_kv_half(2 * t + 3)
                        kv_chunk(t)
                interleave_units(4)
            attn_back(*pairs[-1])
            units.extend([(t, i) for t in range(12, 16) for i in range(NJ)])
            interleave_units(len(units))

            pso_cm.__exit__(None, None, None)
            psa_cm.__exit__(None, None, None)
            fino_cm.__exit__(None, None, None)
            pha_cm.__exit__(None, None, None)
            attnp_cm.__exit__(None, None, None)

    nc.compile()
    _NC_CACHE["nc"] = nc
    return nc


def prep_core_inputs(hidden_states, encoder_hidden_states, id_embedding,
                     Wq, Wk, Wv, Wid_k, Wid_v, Wout, bout):
    """Host-side sharding / layout prep.  Returns list of 8 in_maps."""
    f = np.float32
    h16 = np.float16
    hidden_states = np.asarray(hidden_states, f)
    encoder_hidden_states = np.asarray(encoder_hidden_states, f)
    id_embedding = np.asarray(id_embedding, f)
    Wq = np.asarray(Wq, f)
    Wout = np.asarray(Wout, f)
    Wk, Wv = np.asarray(Wk, f), np.asarray(Wv, f)
    Wid_k, Wid_v = np.asarray(Wid_k, f), np.asarray(Wid_v, f)
    boutb = np.ascontiguousarray(np.broadcast_to(np.asarray(bout, f), (P, H)))

    # packed weight layouts: one contiguous DMA per group
    # wqp[j, p, i*128+c] = Wq[i*128+p, j*128+c]
    wqp = np.ascontiguousarray(
        Wq.reshape(NJ, P, NJ, P).transpose(2, 1, 0, 3)
        .reshape(NJ, P, NJ * P).astype(h16))
    # kv weight half-slabs: [2t+h][p, i*512+c] = W[(8h+i)*128+p, 512n+c]
    wkv = np.concatenate([Wk, Wv], axis=1).reshape(NI, P, 5, 512)
    widkv = np.concatenate([Wid_k, Wid_v], axis=1).reshape(NI, P, 5, 512)
    wkvp = np.empty((20, P, 8 * 512), h16)
    for t, (proj, n) in enumerate(KV_PLAN):
        src = wkv if proj == 0 else widkv
        for h in range(2):
            wkvp[2 * t + h] = (src[8 * h:8 * h + 8, :, n, :]
                               .transpose(1, 0, 2).reshape(P, 8 * 512))

    wout16 = np.ascontiguousarray(Wout.astype(h16))
    in_maps = []
    for core in range(NCORES):
        b, hf = divmod(core, 2)
        hsT = np.ascontiguousarray(
            hidden_states[b, hf * SC:(hf + 1) * SC, :].T.astype(h16)
        ).reshape(NJ, P, SC)
        xkvT = np.zeros((C, LP), h16)                                          # [C, 128]
        xkvT[:, :TE] = encoder_hidden_states[b].T
        xkvT[:, GAP1:] = id_embedding[b % 2].T
        # [i, p, l] -> [p, i*128+l]
        xkvTp = np.ascontiguousarray(
            xkvT.reshape(NI, P, LP).transpose(1, 0, 2).reshape(P, NI * LP))
        in_maps.append({
            "hsT": hsT, "xkvTp": xkvTp, "wqp": wqp, "wkvp": wkvp,
            "woutT": wout16, "boutb": boutb,
        })
    return in_maps


def kernel(hidden_states, encoder_hidden_states, id_embedding,
           Wq, Wk, Wv, Wid_k, Wid_v, Wout, bout, _trace=False):
    _ensure_axon_hooks()
    from concourse.bass_utils import run_bass_kernel_spmd

    nc = build_nc()
    in_maps = prep_core_inputs(hidden_states, encoder_hidden_states, id_embedding,
                               Wq, Wk, Wv, Wid_k, Wid_v, Wout, bout)
    kwargs = {}
    if _trace:
        import concourse.bass_utils as bu
        bu.upload_artifacts = lambda tmpdir: f"local://{tmpdir}"
        kwargs["trace"] = True
    res = run_bass_kernel_spmd(nc, in_maps, core_ids=list(range(NCORES)), **kwargs)

    outp = np.empty((B, S, H), np.float32)
    for core in range(NCORES):
        b, hf = divmod(core, 2)
        outp[b, hf * SC:(hf + 1) * SC, :] = res.results[core]["out"].astype(np.float32)
    if _trace:
        kernel.last_exec_time_ns = res.exec_time_ns
        kernel.last_results = res
    return outp
